# revision 8
# baseline (speedup 1.0000x reference)
"""Trainium2 Bass kernel for a 5-layer LSTM classifier (PaperLSTMClassifier).

Model: B=1024, T=1024, H=64, L=5 layers, V=32 vocab, variable lengths.
Strategy: data-parallel over 8 NeuronCores (128 batch columns each).

Device-side design:
  - State kept feature-major [H, B] in SBUF, in lanes 64-127 of per-layer
    state tiles hh[l] (Hs) and st[l] (Dc). Gate matmuls are split into two
    K=64 matmuls accumulating in PSUM: Wx^T.T @ input + Wh^T.T @ Hs_l, each
    reading the producer's tile in place (no data movement on the
    recurrence chain).
  - Re-parameterization: store Hs = h/2 and Dc = c/2. All four gates are
    Sigmoid only (tanh(x) = 2*sigmoid(2x)-1 folded into host-scaled
    weights):
       [i; f]  = sig(mm + [b_i; b_f])      (lanes: i 0-63, f 64-127)
       [g'; o] = sig(mm + [2 b_g; b_o])    (lanes: g' 0-63, o 64-127)
       iG = (g' - 1/2) * i                 lanes 0-63   (DVE)
       fD = f * Dc                         lanes 64-127 (gpsimd)
       Dc' = iG + fD  via PE pair-sum matmul (cross-lane)  -> PSUM 64-127
       u  = sig(4 Dc')                     lanes 64-127 (ACT)
       Hs' = (u - 1/2) * o                 lanes 64-127 (DVE)
    All rescaling folded into host-prepared weights; math is exact.
  - Sequential reference semantics: wave w = timestep w, layers 0..4 in
    order within the wave (layer l consumes Hs_{l-1} of the same step).
    Cross-wave pipelining is discovered by the Tile scheduler.
  - No length masking: columns evolve independently; h4^T is staged every
    wave (PE transpose -> PSUM -> SBUF stage) and written per-iteration to
    a DRAM ring Y[nit, BL, U, HD]; one indirect DMA gathers row
    (t=len[b]-1, b) at the end, then the head matmul runs on device.
"""

import numpy as np

B, T, HD, L, V = 1024, 1024, 64, 5, 32
NCORES = 8
BL = B // NCORES          # 128 batch columns per core
CAT = 2 * HD              # 128
U = 16                    # waves per For_i iteration
NIT = T // U              # 64 iterations
assert U * NIT == T

_COMPILED = {}


def _build(u, nit):
    from contextlib import ExitStack

    import concourse.bass as bass
    import concourse.tile as tile
    from concourse import bacc, mybir
    from concourse.alu_op_type import AluOpType
    from concourse.bass import ds

    f32 = mybir.dt.float32
    i32 = mybir.dt.int32
    SIG = mybir.ActivationFunctionType.Sigmoid
    SUB = AluOpType.subtract
    MUL = AluOpType.mult

    nc = bacc.Bacc("TRN2", target_bir_lowering=False, debug=False)

    # weights: [L, 2(gate-pair), 2(x/h part), 64, 128] lhsT blocks
    wmm_d = nc.dram_tensor("wmm", [L, 2, 2, 64, 128], f32, kind="ExternalInput")
    bias_d = nc.dram_tensor("biasv", [128, 2 * L], f32, kind="ExternalInput")
    hs0_d = nc.dram_tensor("hs0", [L, 64, BL], f32, kind="ExternalInput")
    exm_d = nc.dram_tensor("exm", [nit, HD, u * BL], f32, kind="ExternalInput")
    gidx_d = nc.dram_tensor("gidx", [BL, 1], i32, kind="ExternalInput")
    headw_d = nc.dram_tensor("headw", [HD, 1], f32, kind="ExternalInput")
    perm_d = nc.dram_tensor("permm", [3, 128, 128], f32, kind="ExternalInput")
    out_d = nc.dram_tensor("out", [BL, 1], f32, kind="ExternalOutput")
    y_d = nc.dram_tensor("yring", [nit, BL, u, HD], f32)

    with tile.TileContext(nc) as tc, ExitStack() as ctx:
        const = ctx.enter_context(tc.tile_pool(name="const", bufs=1))
        psg = ctx.enter_context(
            tc.tile_pool(name="psg", bufs=4, space=bass.MemorySpace.PSUM)
        )
        pst = ctx.enter_context(
            tc.tile_pool(name="pst", bufs=2, space=bass.MemorySpace.PSUM)
        )
        gates = ctx.enter_context(tc.tile_pool(name="gates", bufs=6))
        prods = ctx.enter_context(tc.tile_pool(name="prods", bufs=6))
        upool = ctx.enter_context(tc.tile_pool(name="upool", bufs=6))
        stages = ctx.enter_context(tc.tile_pool(name="stages", bufs=2))

        # --- constants ---
        # x-part weights: layer 0 at lanes 0-63 (reads ex), layers>=1 at
        # lanes 64-127 (read Hs_{l-1} in place). h-part always lanes 64-127.
        wx = const.tile([128, L, 2, 128], f32, tag="wx")
        wh = const.tile([128, L, 2, 128], f32, tag="wh")
        for l in range(L):
            for g in range(2):
                xbase = slice(0, 64) if l == 0 else slice(64, 128)
                nc.sync.dma_start(wx[xbase, l, g, :], wmm_d[l, g, 0])
                nc.sync.dma_start(wh[64:128, l, g, :], wmm_d[l, g, 1])
        bsb = const.tile([128, 2 * L], f32, tag="bsb")
        nc.sync.dma_start(bsb[:], bias_d[:])
        ident = const.tile([128, 128], f32, tag="ident")
        nc.sync.dma_start(ident[:], perm_d[0])
        perm = const.tile([128, 128], f32, tag="perm")
        nc.sync.dma_start(perm[:], perm_d[1])
        p2sb = const.tile([128, 128], f32, tag="p2sb")
        nc.sync.dma_start(p2sb[:], perm_d[2])
        gidx_sb = const.tile([BL, 1], i32, tag="gidx")
        nc.sync.dma_start(gidx_sb[:], gidx_d[:])
        headw_sb = const.tile([HD, 1], f32, tag="headw")
        nc.sync.dma_start(headw_sb[:], headw_d[:])

        # --- persistent state: Hs in hh[l][64:128], Dc in st[l][64:128] ---
        hh = []
        st = []
        for l in range(L):
            h_t = const.tile([128, BL], f32, tag=f"hh{l}")
            nc.vector.memset(h_t[:], 0.0)
            nc.sync.dma_start(h_t[64:128, :], hs0_d[l])
            hh.append(h_t)
            s_t = const.tile([128, BL], f32, tag=f"st{l}")
            nc.vector.memset(s_t[64:128, :], 0.0)
            st.append(s_t)

        def layer_step(l, ex_rhs):
            # gates PSUM: [i;f] cols 0-127, [g';o] cols 128-255
            ps = psg.tile([128, 256], f32, tag="ps", bufs=4)
            for g in range(2):
                cols = slice(128 * g, 128 * (g + 1))
                if l == 0:
                    nc.tensor.matmul(
                        ps[:, cols], wx[0:64, l, g, :], ex_rhs,
                        start=True, stop=False,
                    )
                else:
                    nc.tensor.matmul(
                        ps[:, cols], wx[64:128, l, g, :], hh[l - 1][64:128, :],
                        start=True, stop=False,
                    )
                nc.tensor.matmul(
                    ps[:, cols], wh[64:128, l, g, :], hh[l][64:128, :],
                    start=False, stop=True,
                )
            sbif = gates.tile([128, BL], f32, tag="sbif")  # [i; f]
            sbgo = gates.tile([128, BL], f32, tag="sbgo")  # [g'; o]
            nc.scalar.activation(
                sbif[:], ps[:, 0:128], SIG, bias=bsb[:, 2 * l : 2 * l + 1]
            )
            nc.scalar.activation(
                sbgo[:], ps[:, 128:256], SIG, bias=bsb[:, 2 * l + 1 : 2 * l + 2]
            )
            pr = prods.tile([128, BL], f32, tag="pr")
            # lanes 0-63:  iG = (g' - 1/2) * i
            nc.vector.scalar_tensor_tensor(
                pr[0:64, :], sbgo[0:64, :], 0.5, sbif[0:64, :], SUB, MUL
            )
            # lanes 64-127: fD = f * Dc
            nc.gpsimd.tensor_tensor(
                pr[64:128, :], sbif[64:128, :], st[l][64:128, :], MUL
            )
            # PE pair-sum: pd[64+j] = pr[j] + pr[64+j] = Dc'
            pd = psg.tile([128, BL], f32, tag="pd", bufs=2)
            nc.tensor.matmul(pd[:], p2sb[:], pr[:], start=True, stop=True)
            nc.vector.tensor_copy(st[l][64:128, :], pd[64:128, :])
            uu = upool.tile([128, BL], f32, tag="uu")
            nc.scalar.activation(
                uu[64:128, :], pd[64:128, :], SIG, bias=0.0, scale=4.0
            )
            # Hs' = (u - 1/2) * o
            nc.vector.scalar_tensor_tensor(
                hh[l][64:128, :], uu[64:128, :], 0.5, sbgo[64:128, :], SUB, MUL
            )

        with tc.For_i(
            0, nit, 1,
            hint_engines=(
                mybir.EngineType.Activation,
                mybir.EngineType.DVE,
                mybir.EngineType.PE,
            ),
        ) as it:
            exstage = stages.tile([64, u * BL], f32, tag="exstage", bufs=2)
            nc.sync.dma_start(exstage[:], exm_d[ds(it, 1), :, :])
            ystage = stages.tile([BL, u, HD], f32, tag="ystage", bufs=2)
            for j in range(u):
                for l in range(L):
                    layer_step(l, exstage[:, j * BL : (j + 1) * BL])
                # tap h4: transpose via perm (cols 0-63 = Hs4^T)
                pt = pst.tile([BL, 128], f32, tag="pt", bufs=2)
                nc.tensor.transpose(pt[:], hh[L - 1][:, :], perm[:])
                nc.scalar.copy(ystage[:, j, :], pt[:, 0:HD])
            nc.sync.dma_start(y_d[ds(it, 1), :, :, :], ystage[:])

        # --- epilogue: gather h4 at t=len-1, head matmul ---
        tc.strict_bb_all_engine_barrier()
        g4 = const.tile([BL, HD], f32, tag="g4")
        nc.gpsimd.indirect_dma_start(
            out=g4[:],
            out_offset=None,
            in_=y_d[:].rearrange("a b c d -> (a b c) d"),
            in_offset=bass.IndirectOffsetOnAxis(ap=gidx_sb[:, 0:1], axis=0),
        )
        ptr = pst.tile([HD, BL], f32, tag="pt", bufs=2)
        nc.tensor.transpose(ptr[:], g4[:], ident[:])
        hsb = const.tile([HD, BL], f32, tag="hsb")
        nc.scalar.copy(hsb[:], ptr[:])
        po = pst.tile([BL, 1], f32, tag="pt", bufs=2)
        nc.tensor.matmul(po[:], hsb[:], headw_sb[:], start=True, stop=True)
        osb = const.tile([BL, 1], f32, tag="osb")
        nc.scalar.copy(osb[:], po[:])
        nc.sync.dma_start(out_d[:], osb[:])

    nc.compile()
    return nc


def _prep_host(x, lengths, emb, W_i, W_f, W_g, W_o, b_i, b_f, b_g, b_o,
               init_h, head_w, head_b, u, nit):
    """Build per-core input maps."""
    x = np.asarray(x, dtype=np.int64)
    lengths = np.asarray(lengths, dtype=np.int64)
    emb = np.asarray(emb, dtype=np.float32)
    t_total = u * nit

    # lhsT blocks [K=64, M=128]: x-part scale 1 (l=0) or 2 (l>0, input is
    # Hs_{l-1}); h-part scale 2 (state is Hs). g-gate rows additionally x2.
    wmm = np.empty((L, 2, 2, 64, 128), dtype=np.float32)
    biasv = np.empty((128, 2 * L), dtype=np.float32)
    for l in range(L):
        sx = 1.0 if l == 0 else 2.0
        a_if = np.concatenate([W_i[l], W_f[l]], axis=0)          # [128, CAT]
        a_go = np.concatenate([2.0 * W_g[l], W_o[l]], axis=0)
        for g, a in enumerate((a_if, a_go)):
            wmm[l, g, 0] = (a[:, :HD] * sx).T.astype(np.float32)
            wmm[l, g, 1] = (a[:, HD:] * 2.0).T.astype(np.float32)
        biasv[:, 2 * l] = np.concatenate([b_i[l], b_f[l]])
        biasv[:, 2 * l + 1] = np.concatenate([2.0 * b_g[l], b_o[l]])

    hs0_1 = (np.tanh(np.asarray(init_h, dtype=np.float32)) / 2.0)  # [L, HD]
    headw = (2.0 * np.asarray(head_w, dtype=np.float32)[0])[:, None]

    p2m = np.zeros((128, 128), dtype=np.float32)
    for jj in range(64):
        p2m[jj, 64 + jj] = 1.0
        p2m[64 + jj, 64 + jj] = 1.0
    permm = np.stack(
        [np.eye(128, dtype=np.float32),
         np.roll(np.eye(128, dtype=np.float32), 64, axis=0),
         p2m]
    )

    ex_all = emb[x]  # [B, T_model, H] float32

    in_maps = []
    for c in range(NCORES):
        sl = slice(c * BL, (c + 1) * BL)
        ex_c = ex_all[sl].transpose(1, 2, 0).astype(np.float32)  # [T, H, BL]
        # exm[i, :, j*BL:(j+1)*BL] = ex at t = i*u + j
        exm = np.ascontiguousarray(
            ex_c[:t_total].reshape(nit, u, HD, BL)
            .transpose(0, 2, 1, 3)
            .reshape(nit, HD, u * BL)
        )
        hs0 = np.repeat(hs0_1[:, :, None], BL, axis=2).astype(np.float32)
        t_b = lengths[sl].astype(np.int64) - 1  # in [0, T-1]
        rows = ((t_b // u) * BL + np.arange(BL)) * u + (t_b % u)
        in_maps.append(
            {
                "wmm": wmm,
                "biasv": biasv,
                "hs0": hs0,
                "exm": exm,
                "gidx": rows.astype(np.int32)[:, None],
                "headw": headw,
                "permm": permm,
            }
        )
    return in_maps


def kernel(x, lengths, emb, W_i, W_f, W_g, W_o, b_i, b_f, b_g, b_o,
           init_h, head_w, head_b, _trace=False):
    from concourse.bass_utils import run_bass_kernel_spmd

    key = (U, NIT)
    if key not in _COMPILED:
        _COMPILED[key] = _build(U, NIT)
    nc = _COMPILED[key]

    in_maps = _prep_host(
        x, lengths, emb, W_i, W_f, W_g, W_o, b_i, b_f, b_g, b_o,
        init_h, head_w, head_b, U, NIT,
    )
    res = run_bass_kernel_spmd(nc, in_maps, list(range(NCORES)), trace=_trace)
    outs = [res.results[c]["out"][:, 0] for c in range(NCORES)]
    logits = np.concatenate(outs).astype(np.float32) + np.float32(
        np.asarray(head_b).reshape(-1)[0]
    )
    if _trace:
        kernel._last_exec_time_ns = res.exec_time_ns
        kernel._last_profile = res.profile_json
    return logits


# revision 9
# speedup vs baseline: 1.1854x; 1.1854x over previous
"""Trainium2 Bass kernel for a 5-layer LSTM classifier (PaperLSTMClassifier).

Model: B=1024, T=1024, H=64, L=5 layers, V=32 vocab, variable lengths.
Strategy: data-parallel over 8 NeuronCores (128 batch columns each).

Device-side design:
  - State kept feature-major [H, B] in SBUF, in lanes 64-127 of per-layer
    state tiles hh[l] (Hs) and st[l] (Dc). Gate matmuls are split into two
    K=64 matmuls accumulating in PSUM: Wx^T.T @ input + Wh^T.T @ Hs_l, each
    reading the producer's tile in place (no data movement on the
    recurrence chain).
  - Re-parameterization: store Hs = h/2 and Dc = c/2. All four gates are
    Sigmoid only (tanh(x) = 2*sigmoid(2x)-1 folded into host-scaled
    weights):
       [i; f]  = sig(mm + [b_i; b_f])      (lanes: i 0-63, f 64-127)
       [g'; o] = sig(mm + [2 b_g; b_o])    (lanes: g' 0-63, o 64-127)
       iG = (g' - 1/2) * i                 lanes 0-63   (DVE)
       fD = f * Dc                         lanes 64-127 (gpsimd)
       Dc' = iG + fD  via PE pair-sum matmul (cross-lane)  -> PSUM 64-127
       u  = sig(4 Dc')                     lanes 64-127 (ACT)
       Hs' = (u - 1/2) * o                 lanes 64-127 (DVE)
    All rescaling folded into host-prepared weights; math is exact.
  - Sequential reference semantics: wave w = timestep w, layers 0..4 in
    order within the wave (layer l consumes Hs_{l-1} of the same step).
    Cross-wave pipelining is discovered by the Tile scheduler.
  - No length masking: columns evolve independently; h4^T is staged every
    wave (PE transpose -> PSUM -> SBUF stage) and written per-iteration to
    a DRAM ring Y[nit, BL, U, HD]; one indirect DMA gathers row
    (t=len[b]-1, b) at the end, then the head matmul runs on device.
"""

import numpy as np

B, T, HD, L, V = 1024, 1024, 64, 5, 32
NCORES = 8
BL = B // NCORES          # 128 batch columns per core
CAT = 2 * HD              # 128
U = 16                    # waves per For_i iteration
NIT = T // U              # 64 iterations
assert U * NIT == T

_COMPILED = {}


def _build(u, nit):
    from contextlib import ExitStack

    import concourse.bass as bass
    import concourse.tile as tile
    from concourse import bacc, mybir
    from concourse.alu_op_type import AluOpType
    from concourse.bass import ds

    f32 = mybir.dt.float32
    i32 = mybir.dt.int32
    SIG = mybir.ActivationFunctionType.Sigmoid
    SUB = AluOpType.subtract
    MUL = AluOpType.mult

    nc = bacc.Bacc("TRN2", target_bir_lowering=False, debug=False)

    # weights: [L, 2(gate-pair), 2(x/h part), 64, 128] lhsT blocks
    wmm_d = nc.dram_tensor("wmm", [L, 2, 2, 64, 128], f32, kind="ExternalInput")
    bias_d = nc.dram_tensor("biasv", [128, 2 * L], f32, kind="ExternalInput")
    hs0_d = nc.dram_tensor("hs0", [L, 64, BL], f32, kind="ExternalInput")
    exm_d = nc.dram_tensor("exm", [nit, HD, u * BL], f32, kind="ExternalInput")
    gidx_d = nc.dram_tensor("gidx", [BL, 1], i32, kind="ExternalInput")
    headw_d = nc.dram_tensor("headw", [HD, 1], f32, kind="ExternalInput")
    perm_d = nc.dram_tensor("permm", [2, 128, 128], f32, kind="ExternalInput")
    out_d = nc.dram_tensor("out", [BL, 1], f32, kind="ExternalOutput")
    y_d = nc.dram_tensor("yring", [nit, BL, u, HD], f32)

    with tile.TileContext(nc) as tc, ExitStack() as ctx:
        const = ctx.enter_context(tc.tile_pool(name="const", bufs=1))
        psg = ctx.enter_context(
            tc.tile_pool(name="psg", bufs=4, space=bass.MemorySpace.PSUM)
        )
        pst = ctx.enter_context(
            tc.tile_pool(name="pst", bufs=2, space=bass.MemorySpace.PSUM)
        )
        gates = ctx.enter_context(tc.tile_pool(name="gates", bufs=6))
        prods = ctx.enter_context(tc.tile_pool(name="prods", bufs=6))
        upool = ctx.enter_context(tc.tile_pool(name="upool", bufs=6))
        stages = ctx.enter_context(tc.tile_pool(name="stages", bufs=2))

        # --- constants ---
        # x-part weights: layer 0 at lanes 0-63 (reads ex), layers>=1 at
        # lanes 64-127 (read Hs_{l-1} in place). h-part always lanes 64-127.
        wx = const.tile([128, L, 2, 128], f32, tag="wx")
        wh = const.tile([128, L, 2, 128], f32, tag="wh")
        for l in range(L):
            for g in range(2):
                nc.sync.dma_start(wx[0:64, l, g, :], wmm_d[l, g, 0])
                nc.sync.dma_start(wh[0:64, l, g, :], wmm_d[l, g, 1])
        bsb = const.tile([128, 2 * L], f32, tag="bsb")
        nc.sync.dma_start(bsb[:], bias_d[:])
        ident = const.tile([128, 128], f32, tag="ident")
        nc.sync.dma_start(ident[:], perm_d[0])
        p2sb = const.tile([128, 128], f32, tag="p2sb")
        nc.sync.dma_start(p2sb[:], perm_d[1])
        gidx_sb = const.tile([BL, 1], i32, tag="gidx")
        nc.sync.dma_start(gidx_sb[:], gidx_d[:])
        headw_sb = const.tile([HD, 1], f32, tag="headw")
        nc.sync.dma_start(headw_sb[:], headw_d[:])

        # --- persistent state: Hs in hh[l][0:64], Dc in st[l][0:64] ---
        hh = []
        st = []
        for l in range(L):
            h_t = const.tile([128, BL], f32, tag=f"hh{l}")
            nc.vector.memset(h_t[:], 0.0)
            nc.sync.dma_start(h_t[0:64, :], hs0_d[l])
            hh.append(h_t)
            s_t = const.tile([64, BL], f32, tag=f"st{l}")
            nc.vector.memset(s_t[:], 0.0)
            st.append(s_t)

        def layer_step(l, ex_rhs):
            # gates PSUM: [f;i] cols 0-127, [o;g'] cols 128-255
            ps = psg.tile([128, 256], f32, tag="ps", bufs=4)
            for g in range(2):
                cols = slice(128 * g, 128 * (g + 1))
                if l == 0:
                    nc.tensor.matmul(
                        ps[:, cols], wx[0:64, l, g, :], ex_rhs,
                        start=True, stop=False,
                    )
                else:
                    nc.tensor.matmul(
                        ps[:, cols], wx[0:64, l, g, :], hh[l - 1][0:64, :],
                        start=True, stop=False,
                    )
                nc.tensor.matmul(
                    ps[:, cols], wh[0:64, l, g, :], hh[l][0:64, :],
                    start=False, stop=True,
                )
            sbfi = gates.tile([128, BL], f32, tag="sbfi")  # [f; i]
            sbog = gates.tile([128, BL], f32, tag="sbog")  # [o; g']
            nc.scalar.activation(
                sbfi[:], ps[:, 0:128], SIG, bias=bsb[:, 2 * l : 2 * l + 1]
            )
            nc.scalar.activation(
                sbog[:], ps[:, 128:256], SIG, bias=bsb[:, 2 * l + 1 : 2 * l + 2]
            )
            pr = prods.tile([128, BL], f32, tag="pr")
            # lanes 0-63:  fD = f * Dc
            nc.gpsimd.tensor_tensor(
                pr[0:64, :], sbfi[0:64, :], st[l][:, :], MUL
            )
            # lanes 64-127: iG = (g' - 1/2) * i
            nc.vector.scalar_tensor_tensor(
                pr[64:128, :], sbog[64:128, :], 0.5, sbfi[64:128, :], SUB, MUL
            )
            # PE pair-sum: pd[j] = pr[j] + pr[64+j] = Dc'  (lanes 0-63)
            pd = psg.tile([128, BL], f32, tag="pd", bufs=2)
            nc.tensor.matmul(pd[:], p2sb[:], pr[:], start=True, stop=True)
            nc.vector.tensor_copy(st[l][:, :], pd[0:64, :])
            uu = upool.tile([64, BL], f32, tag="uu")
            nc.scalar.activation(
                uu[:, :], pd[0:64, :], SIG, bias=0.0, scale=4.0
            )
            # Hs' = (u - 1/2) * o   (lanes 0-63)
            nc.vector.scalar_tensor_tensor(
                hh[l][0:64, :], uu[:, :], 0.5, sbog[0:64, :], SUB, MUL
            )

        with tc.For_i(
            0, nit, 1,
            hint_engines=(
                mybir.EngineType.Activation,
                mybir.EngineType.DVE,
                mybir.EngineType.PE,
            ),
        ) as it:
            exstage = stages.tile([64, u * BL], f32, tag="exstage", bufs=2)
            nc.sync.dma_start(exstage[:], exm_d[ds(it, 1), :, :])
            ystage = stages.tile([BL, u, HD], f32, tag="ystage", bufs=2)
            for j in range(u):
                for l in range(L):
                    layer_step(l, exstage[:, j * BL : (j + 1) * BL])
                # tap h4: transpose via perm (cols 0-63 = Hs4^T)
                pt = pst.tile([BL, 128], f32, tag="pt", bufs=2)
                nc.tensor.transpose(pt[:], hh[L - 1][:, :], ident[:])
                nc.scalar.copy(ystage[:, j, :], pt[:, 0:HD])
            nc.sync.dma_start(y_d[ds(it, 1), :, :, :], ystage[:])

        # --- epilogue: gather h4 at t=len-1, head matmul ---
        tc.strict_bb_all_engine_barrier()
        g4 = const.tile([BL, HD], f32, tag="g4")
        nc.gpsimd.indirect_dma_start(
            out=g4[:],
            out_offset=None,
            in_=y_d[:].rearrange("a b c d -> (a b c) d"),
            in_offset=bass.IndirectOffsetOnAxis(ap=gidx_sb[:, 0:1], axis=0),
        )
        ptr = pst.tile([HD, BL], f32, tag="pt", bufs=2)
        nc.tensor.transpose(ptr[:], g4[:], ident[:])
        hsb = const.tile([HD, BL], f32, tag="hsb")
        nc.scalar.copy(hsb[:], ptr[:])
        po = pst.tile([BL, 1], f32, tag="pt", bufs=2)
        nc.tensor.matmul(po[:], hsb[:], headw_sb[:], start=True, stop=True)
        osb = const.tile([BL, 1], f32, tag="osb")
        nc.scalar.copy(osb[:], po[:])
        nc.sync.dma_start(out_d[:], osb[:])

    nc.compile()
    return nc


def _prep_host(x, lengths, emb, W_i, W_f, W_g, W_o, b_i, b_f, b_g, b_o,
               init_h, head_w, head_b, u, nit):
    """Build per-core input maps."""
    x = np.asarray(x, dtype=np.int64)
    lengths = np.asarray(lengths, dtype=np.int64)
    emb = np.asarray(emb, dtype=np.float32)
    t_total = u * nit

    # lhsT blocks [K=64, M=128]: x-part scale 1 (l=0) or 2 (l>0, input is
    # Hs_{l-1}); h-part scale 2 (state is Hs). g-gate rows additionally x2.
    wmm = np.empty((L, 2, 2, 64, 128), dtype=np.float32)
    biasv = np.empty((128, 2 * L), dtype=np.float32)
    for l in range(L):
        sx = 1.0 if l == 0 else 2.0
        a_fi = np.concatenate([W_f[l], W_i[l]], axis=0)          # [128, CAT]
        a_og = np.concatenate([W_o[l], 2.0 * W_g[l]], axis=0)
        for g, a in enumerate((a_fi, a_og)):
            wmm[l, g, 0] = (a[:, :HD] * sx).T.astype(np.float32)
            wmm[l, g, 1] = (a[:, HD:] * 2.0).T.astype(np.float32)
        biasv[:, 2 * l] = np.concatenate([b_f[l], b_i[l]])
        biasv[:, 2 * l + 1] = np.concatenate([b_o[l], 2.0 * b_g[l]])

    hs0_1 = (np.tanh(np.asarray(init_h, dtype=np.float32)) / 2.0)  # [L, HD]
    headw = (2.0 * np.asarray(head_w, dtype=np.float32)[0])[:, None]

    p2m = np.zeros((128, 128), dtype=np.float32)
    for jj in range(64):
        p2m[jj, jj] = 1.0
        p2m[64 + jj, jj] = 1.0
    permm = np.stack([np.eye(128, dtype=np.float32), p2m])

    ex_all = emb[x]  # [B, T_model, H] float32

    in_maps = []
    for c in range(NCORES):
        sl = slice(c * BL, (c + 1) * BL)
        ex_c = ex_all[sl].transpose(1, 2, 0).astype(np.float32)  # [T, H, BL]
        # exm[i, :, j*BL:(j+1)*BL] = ex at t = i*u + j
        exm = np.ascontiguousarray(
            ex_c[:t_total].reshape(nit, u, HD, BL)
            .transpose(0, 2, 1, 3)
            .reshape(nit, HD, u * BL)
        )
        hs0 = np.repeat(hs0_1[:, :, None], BL, axis=2).astype(np.float32)
        t_b = lengths[sl].astype(np.int64) - 1  # in [0, T-1]
        rows = ((t_b // u) * BL + np.arange(BL)) * u + (t_b % u)
        in_maps.append(
            {
                "wmm": wmm,
                "biasv": biasv,
                "hs0": hs0,
                "exm": exm,
                "gidx": rows.astype(np.int32)[:, None],
                "headw": headw,
                "permm": permm,
            }
        )
    return in_maps


def kernel(x, lengths, emb, W_i, W_f, W_g, W_o, b_i, b_f, b_g, b_o,
           init_h, head_w, head_b, _trace=False):
    from concourse.bass_utils import run_bass_kernel_spmd

    key = (U, NIT)
    if key not in _COMPILED:
        _COMPILED[key] = _build(U, NIT)
    nc = _COMPILED[key]

    in_maps = _prep_host(
        x, lengths, emb, W_i, W_f, W_g, W_o, b_i, b_f, b_g, b_o,
        init_h, head_w, head_b, U, NIT,
    )
    res = run_bass_kernel_spmd(nc, in_maps, list(range(NCORES)), trace=_trace)
    outs = [res.results[c]["out"][:, 0] for c in range(NCORES)]
    logits = np.concatenate(outs).astype(np.float32) + np.float32(
        np.asarray(head_b).reshape(-1)[0]
    )
    if _trace:
        kernel._last_exec_time_ns = res.exec_time_ns
        kernel._last_profile = res.profile_json
    return logits
